# revision 23
# baseline (speedup 1.0000x reference)
"""GCN VGAE encoder (2-layer GCNConv, mu|logvar fused) on 8 TRN2 cores.

Strategy: shard destination nodes across 8 cores; partition edges by
dst (host-side); replicate weights. Layer-1 node table (x@W1)*dinv is
computed redundantly in full on every core (cheap on PE, kills one
AllGather); layer-2 table requires one fp32 AllGather. Per-edge
aggregation = dma_gather of 256B table rows from HBM (4 SWDGE queues,
small calls) + 0/1-matrix scatter matmuls into PSUM per dst tile.

Math: A_hat = D^-1/2 (A+I) D^-1/2 factorizes, so
    gcn(x, W) = dinv * [ (x@W)*dinv + A @ ((x@W)*dinv) ] + b
with dinv = 1/sqrt(deg+1). Self loop = the own-row term.

Table layout (both layers): [8 cores x 12544 rows x 64 f32] where core
section c row r (tile-padded: r = t*128 + p, 98 tiles, rows >= 12500
are pad) holds node c*12500 + r. Quarters of 25088 rows (= 2 core
sections) keep gather indices within int16. SPMD: the program is
identical on every core; all per-core differences are input data
(xt_own, dinv_own, gidx, dstloc).
"""

import os

import numpy as np

P = 128
N_CORES = 8
S = 12500             # real nodes per core
T = 98                # dst tiles per core
SPAD = T * P          # 12544 padded rows per core
NQ = 4                # table quarters (2 core sections each)
QR = 2 * SPAD         # 25088 rows per quarter
NPAD = N_CORES * SPAD


def _ceil_div(a, b):
    return -(-a // b)


class _Plan:
    """Host-side edge partitioning (SPMD: one common chunk structure =
    max over cores, padded)."""

    def __init__(self, src, dst, cpc):
        self.cpc = cpc

        core = dst // S                     # dst core
        drel = dst - core * S
        tt = drel // P                      # dst tile within core
        loc = (drel % P).astype(np.float32)
        csrc = src // S                     # src core
        q = csrc // 2                       # quarter of the table
        qsrc = ((csrc % 2) * SPAD + (src - csrc * S)).astype(np.int16)

        key = (core * NQ + q) * T + tt
        counts = np.bincount(key, minlength=N_CORES * NQ * T).reshape(
            N_CORES, NQ, T)
        # common run length per (quarter, dst tile): max over cores; runs
        # packed back-to-back per quarter stream (chunks may span tiles).
        self.rl = counts.max(axis=0)                         # [NQ, T]
        self.run_start = np.zeros((NQ, T), np.int64)
        self.run_start[:, 1:] = np.cumsum(self.rl, axis=1)[:, :-1]
        self.NQE = self.rl.sum(axis=1)                       # edges/quarter
        self.NQC = _ceil_div(self.NQE, P)                    # chunks/quarter
        self.NCH = int(self.NQC.sum())

        # order edges by (core, quarter, tile); rank within group
        sidx = np.lexsort((tt, q, core))
        self.sc = core[sidx]
        self.sq = q[sidx]
        self.st = tt[sidx]
        self.sqsrc = qsrc[sidx]
        self.sloc = loc[sidx]
        gkey = (self.sc * NQ + self.sq) * T + self.st
        first = np.r_[True, gkey[1:] != gkey[:-1]]
        gstart = np.flatnonzero(first)
        glen = np.diff(np.r_[gstart, len(gkey)])
        self.rank = np.arange(len(gkey)) - np.repeat(gstart, glen)

        # chunk-part map: tile t consumes chunks j0..j1 of each quarter's
        # stream; each (t, q, j) overlap owns one dstloc column.
        self.tile_ops = []        # [T] -> list of (q, j, cp_col)
        ncp = 0
        self.cp_base = np.zeros(T, np.int64)
        self.ops_before = np.zeros((NQ, T), np.int64)
        self.j0 = self.run_start // P
        for t in range(T):
            ops = []
            self.cp_base[t] = ncp
            acc = 0
            for qq in range(NQ):
                self.ops_before[qq, t] = acc
                r0 = int(self.run_start[qq, t])
                r1 = r0 + int(self.rl[qq, t])
                if r1 == r0:
                    continue
                for j in range(r0 // P, (r1 - 1) // P + 1):
                    ops.append((qq, j, ncp + acc + j - r0 // P))
                acc += (r1 - 1) // P - r0 // P + 1
            ncp += acc
            self.tile_ops.append(ops)
        self.NCP = ncp

        # gather calls per quarter (cpc chunks per call)
        self.ncalls = [_ceil_div(int(c), cpc) for c in self.NQC]
        self.call_col0 = {}
        col = 0
        for qq in range(NQ):
            for k in range(self.ncalls[qq]):
                L = min(cpc, int(self.NQC[qq]) - k * cpc)
                self.call_col0[(qq, k)] = (col, L)
                col += L * 8
        self.IDXCOLS = col

    def core_arrays(self, c, dst):
        """Per-core upload tensors: gather idx [128, IDXCOLS] i16,
        dstloc [128, NCP] f32."""
        cpc = self.cpc
        m_core = self.sc == c
        idx_out = np.zeros((P, self.IDXCOLS), np.int16)
        dl = np.full((self.NCP, P), 255.0, np.float32)
        mloc = self.sloc[m_core]
        mq = self.sq[m_core]
        mt = self.st[m_core]
        mrank = self.rank[m_core]
        msrc = self.sqsrc[m_core]
        pos = self.run_start[mq, mt] + mrank   # position in quarter stream
        cpcol = (self.cp_base[mt] + self.ops_before[mq, mt]
                 + pos // P - self.j0[mq, mt])
        dl[cpcol, pos % P] = mloc
        for qq in range(NQ):
            mm = mq == qq
            arr = np.zeros(int(self.NQC[qq]) * P, np.int16)
            arr[pos[mm]] = msrc[mm]
            for k in range(self.ncalls[qq]):
                c0, L = self.call_col0[(qq, k)]
                seg = arr[k * cpc * P:(k * cpc + L) * P]
                wrapped = seg.reshape(L * 8, 16).T       # [16, L*8]
                idx_out[:, c0:c0 + L * 8] = np.tile(wrapped, (8, 1))
        return idx_out, dl.T.copy()


def _build(plan, d_in, d_h, d_o, repeats=1):
    """Build the SPMD Bass program (identical on every core)."""
    import concourse.mybir as mybir
    import concourse.tile as tile
    from concourse import bacc
    from concourse.masks import make_identity

    F32 = mybir.dt.float32
    BF16 = mybir.dt.bfloat16
    I16 = mybir.dt.int16
    cpc = plan.cpc
    XC = SPAD // 2     # xT load chunk: quarter of a quarter (6272 rows)

    nc = bacc.Bacc("TRN2", target_bir_lowering=False, debug=False,
                   num_devices=N_CORES, num_swdge_queues=4)

    xt_d = nc.dram_tensor("xt", [d_in, NPAD], BF16, kind="ExternalInput")
    xto_d = nc.dram_tensor("xt_own", [d_in, SPAD], BF16,
                           kind="ExternalInput")
    w1_d = nc.dram_tensor("w1", [d_in, P], BF16, kind="ExternalInput")
    wc_d = nc.dram_tensor("wcat", [d_h, P], F32, kind="ExternalInput")
    b1_d = nc.dram_tensor("b1", [d_h], F32, kind="ExternalInput")
    bc_d = nc.dram_tensor("bcat", [d_o], F32, kind="ExternalInput")
    dinv_d = nc.dram_tensor("dinv", [P, N_CORES * T], F32,
                            kind="ExternalInput")
    dvo_d = nc.dram_tensor("dinv_own", [P, T], F32, kind="ExternalInput")
    dl_d = nc.dram_tensor("dstloc", [P, plan.NCP], F32, kind="ExternalInput")
    idx_d = nc.dram_tensor("gidx", [P, plan.IDXCOLS], I16,
                           kind="ExternalInput")
    out_d = nc.dram_tensor("out2", [P, T, d_o], F32, kind="ExternalOutput")

    t1q = [nc.dram_tensor(f"t1q{q}", [QR, P], BF16, kind="Internal")
           for q in range(NQ)]
    z2s_own = nc.dram_tensor("z2s_own", [SPAD, P], BF16, kind="Internal")
    z2s_full = nc.dram_tensor("z2s_full", [NPAD, P], BF16, kind="Internal",
                              addr_space="Shared")
    _priv = bool(os.environ.get("GCN_PRIV"))
    z2s_priv = (nc.dram_tensor("z2s_priv", [NPAD, P], BF16,
                               kind="Internal") if _priv else None)
    rg = [list(range(N_CORES))]

    _stage = int(os.environ.get("GCN_STAGE", "2"))
    _nocoll = bool(os.environ.get("GCN_NOCOLL"))
    _l1agg = os.environ.get("GCN_L1AGG", "1") != "0"
    _gbufs = int(os.environ.get("GCN_GBUFS", "8"))

    from contextlib import ExitStack

    with tile.TileContext(nc, num_cores=N_CORES) as tc, ExitStack() as st:
        cp = st.enter_context(tc.tile_pool(name="consts", bufs=1))
        bigp = st.enter_context(tc.tile_pool(name="big", bufs=1))
        xtp = st.enter_context(tc.tile_pool(name="xt", bufs=2))
        stp = st.enter_context(tc.tile_pool(name="stage", bufs=2))
        htp = st.enter_context(tc.tile_pool(name="ht", bufs=2))
        ohp = st.enter_context(tc.tile_pool(
            name="oh", bufs=int(os.environ.get("GCN_OBUFS", "6"))))
        gp = st.enter_context(tc.tile_pool(name="g", bufs=_gbufs))
        mmp = st.enter_context(tc.tile_pool(name="mm", bufs=2, space="PSUM"))
        aggp = st.enter_context(tc.tile_pool(name="agg", bufs=6,
                                             space="PSUM"))

        # ---- constants ----
        iota_i = cp.tile([P, P], mybir.dt.int32)
        nc.gpsimd.iota(iota_i[:], pattern=[[1, P]], base=0,
                       channel_multiplier=0)
        iota_f = cp.tile([P, P], F32)
        nc.vector.tensor_copy(iota_f[:], iota_i[:])
        ident = cp.tile([P, P], F32)
        make_identity(nc, ident[:])
        ident_bf = cp.tile([P, P], BF16)
        nc.vector.tensor_copy(ident_bf[:], ident[:])
        ones_row = cp.tile([1, P], F32)
        nc.gpsimd.memset(ones_row[:], 1.0)

        w1_sb = cp.tile([d_in, P], BF16)
        nc.sync.dma_start(w1_sb[:], w1_d[:, :])
        wc_sb = cp.tile([d_h, P], F32)
        nc.sync.dma_start(wc_sb[:], wc_d[:, :])
        b1r = cp.tile([1, d_h], F32)
        nc.sync.dma_start(b1r[:], b1_d[None, :])
        bcr = cp.tile([1, d_o], F32)
        nc.sync.dma_start(bcr[:], bc_d[None, :])

        b1bc = cp.tile([P, d_h], F32)
        ps = mmp.tile([P, P], F32, space="PSUM", tag="mm")
        nc.tensor.matmul(ps[:, :d_h], lhsT=ones_row[:], rhs=b1r[:],
                         start=True, stop=True)
        nc.vector.tensor_copy(b1bc[:], ps[:, :d_h])
        bcbc = cp.tile([P, d_o], F32)
        ps = mmp.tile([P, P], F32, space="PSUM", tag="mm")
        nc.tensor.matmul(ps[:, :d_o], lhsT=ones_row[:], rhs=bcr[:],
                         start=True, stop=True)
        nc.vector.tensor_copy(bcbc[:], ps[:, :d_o])

        dinv = cp.tile([P, N_CORES * T], F32)   # full-table dinv
        nc.sync.dma_start(dinv[:], dinv_d[:, :])
        dvo = cp.tile([P, T], F32)              # own dinv
        nc.sync.dma_start(dvo[:], dvo_d[:, :])
        dl_sb = cp.tile([P, plan.NCP], F32)
        nc.sync.dma_start(dl_sb[:], dl_d[:, :])
        idx_sb = cp.tile([P, plan.IDXCOLS], I16)
        nc.sync.dma_start(idx_sb[:], idx_d[:, :])

        acc_sb = bigp.tile([P, T, d_h], F32)
        z2s_sb = acc_sb    # acc is dead per-tile once the h-loop used it
        z2st_g = bigp.tile([P, T, P], BF16)

        OHK = int(os.environ.get("GCN_OHK", "16"))  # chunks per oh build

        def agg_pass(table_dram, acc, d_f):
            """acc[:, t, :] += sum_e onehot(dst) * table[src] (all 4
            quarters), via 4-queue HBM gathers."""
            issued = {}
            ohcache = {}
            callno = [0]

            def get_oh(cpcol):
                # batched one-hot build: one DVE op per OHK chunks, with
                # broadcast (stride-0) APs; built well ahead of the
                # consuming matmuls so the engines stay decoupled.
                B = cpcol // OHK
                if B not in ohcache:
                    from concourse.bass import broadcast_tensor_aps
                    L = min(OHK, plan.NCP - B * OHK)
                    ohb = ohp.tile([P, OHK, P], BF16, tag="ohb")
                    a0, a1 = broadcast_tensor_aps(
                        iota_f[:, None, :],
                        dl_sb[:, B * OHK:B * OHK + L, None])
                    nc.vector.tensor_tensor(ohb[:, :L, :], a0, a1,
                                            mybir.AluOpType.is_equal)
                    ohcache[B] = ohb
                return ohcache[B][:, cpcol - B * OHK, :]

            def get_call(qq, k):
                if (qq, k) not in issued:
                    c0, L = plan.call_col0[(qq, k)]
                    g = gp.tile([P, cpc, P], BF16, tag="g")
                    tab = (table_dram[qq] if isinstance(table_dram, list)
                           else table_dram[qq * QR:(qq + 1) * QR, :])
                    nc.gpsimd.dma_gather(
                        out_ap=g[:, :L, :],
                        in_ap=tab[:, :] if isinstance(table_dram, list)
                        else tab,
                        idxs_ap=idx_sb[:, c0:c0 + L * 8],
                        num_idxs=L * P,
                        num_idxs_reg=L * P,
                        elem_size=P,
                        single_packet=False,
                        queue_num=callno[0] % 4,
                    )
                    callno[0] += 1
                    issued[(qq, k)] = g
                return issued[(qq, k)]

            _noh = os.environ.get("GCN_NOOH")
            _fix = os.environ.get("GCN_FIXOH")
            for t in range(T):
                ops = plan.tile_ops[t]
                if not ops:
                    continue
                if _noh:
                    for i, (qq, j, cpcol) in enumerate(ops):
                        get_call(qq, j // cpc)
                    continue
                psa = aggp.tile([P, d_f], F32, space="PSUM", tag="agg")
                for i, (qq, j, cpcol) in enumerate(ops):
                    g = get_call(qq, j // cpc)
                    col = j % cpc
                    if _fix:
                        oh = ident_bf
                        get_oh(cpcol) if os.environ.get(
                            "GCN_OHCONST") else None
                    elif os.environ.get("GCN_OHONE"):
                        oh = get_oh(0)
                    else:
                        oh = get_oh(cpcol)
                    nc.tensor.matmul(psa[:], lhsT=oh[:],
                                     rhs=g[:, col, :d_f],
                                     start=(i == 0),
                                     stop=(i == len(ops) - 1))
                nc.vector.tensor_tensor(acc[:, t, :], acc[:, t, :],
                                        psa[:], mybir.AluOpType.add)

        for _rep in range(repeats):
            # ---- layer 1: full table computed locally, section-wise ----
            # t1s[r] = (x[r] @ W1) * dinv[r]
            for sec in range(N_CORES):
                stage = stp.tile([P, T, P], BF16, tag="stage")
                for h in range(2):       # two xT chunks per section
                    col0 = sec * SPAD + h * XC       # table row offset
                    xt = xtp.tile([d_in, XC], BF16, tag="xt")
                    nc.sync.dma_start(xt[:], xt_d[:, col0:col0 + XC])
                    for t in range(XC // P):
                        tc_glob = col0 // P + t      # global tile index
                        st_loc = h * (XC // P) + t   # tile within stage
                        psm = mmp.tile([P, P], F32, space="PSUM", tag="mm")
                        nc.tensor.matmul(
                            psm[:], lhsT=xt[:, t * P:(t + 1) * P],
                            rhs=w1_sb[:], start=True, stop=True)
                        nc.vector.tensor_scalar(
                            stage[:, st_loc, :], psm[:],
                            dinv[:, tc_glob:tc_glob + 1], None,
                            mybir.AluOpType.mult)
                nc.sync.dma_start(
                    t1q[sec // 2].rearrange("(q p) f -> p q f", p=P)[
                        :, (sec % 2) * T:(sec % 2 + 1) * T, :],
                    stage[:])

            # ---- seed acc with own (x@W1)*dinv section ----
            for h in range(2):
                xt = xtp.tile([d_in, XC], BF16, tag="xt")
                nc.sync.dma_start(xt[:], xto_d[:, h * XC:(h + 1) * XC])
                for t in range(XC // P):
                    tl = h * (XC // P) + t
                    psm = mmp.tile([P, P], F32, space="PSUM", tag="mm")
                    nc.tensor.matmul(
                        psm[:], lhsT=xt[:, t * P:(t + 1) * P],
                        rhs=w1_sb[:], start=True, stop=True)
                    nc.vector.tensor_scalar(
                        acc_sb[:, tl, :], psm[:, :d_h],
                        dvo[:, tl:tl + 1], None, mybir.AluOpType.mult)

            if _stage >= 1 and _l1agg:
                agg_pass(t1q, acc_sb, d_h)
            if os.environ.get("GCN_DUP_L1"):
                agg_pass(t1q, acc_sb, d_h)

            # ---- h = relu(acc * dinv_own + b1); z2s = (h @ Wcat)*dinv ----
            z2st = z2st_g
            for t in range(T):
                dv = dvo[:, t:t + 1]
                nc.vector.scalar_tensor_tensor(
                    acc_sb[:, t, :], acc_sb[:, t, :], dv, b1bc[:],
                    mybir.AluOpType.mult, mybir.AluOpType.add)
                nc.scalar.activation(acc_sb[:, t, :], acc_sb[:, t, :],
                                     mybir.ActivationFunctionType.Relu)
                pst = mmp.tile([P, P], F32, space="PSUM", tag="mm")
                nc.tensor.transpose(pst[:d_h, :], acc_sb[:, t, :], ident[:])
                hT = htp.tile([d_h, P], F32, tag="ht")
                nc.vector.tensor_copy(hT[:], pst[:d_h, :])
                psm = mmp.tile([P, P], F32, space="PSUM", tag="mm")
                nc.tensor.matmul(psm[:], lhsT=hT[:], rhs=wc_sb[:],
                                 start=True, stop=True)
                nc.vector.tensor_scalar(z2s_sb[:, t, :], psm[:, :d_o],
                                        dv, None, mybir.AluOpType.mult)
                nc.vector.tensor_scalar(z2st[:, t, :], psm[:],
                                        dv, None, mybir.AluOpType.mult)
            nc.sync.dma_start(
                z2s_own.rearrange("(t p) f -> p t f", p=P)[:, :, :],
                z2st[:])

            if _nocoll:
                nc.sync.dma_start(z2s_full[0:SPAD, :], z2s_own[:, :])
            else:
                nc.gpsimd.collective_compute(
                    "AllGather", mybir.AluOpType.bypass, replica_groups=rg,
                    ins=[z2s_own[:, :].opt()], outs=[z2s_full[:, :].opt()])

            # acc2 = z2s_sb (own section seed), aggregate in place
            if _stage >= 2:
                if _priv:
                    nc.sync.dma_start(z2s_priv[:, :], z2s_full[:, :])
                    agg_pass(z2s_priv, z2s_sb, d_o)
                else:
                    agg_pass(z2s_full, z2s_sb, d_o)

            # ---- out = acc2 * dinv_own + bcat (into acc_sb, then store) --
            for t in range(T):
                nc.vector.scalar_tensor_tensor(
                    acc_sb[:, t, :], z2s_sb[:, t, :],
                    dvo[:, t:t + 1], bcbc[:],
                    mybir.AluOpType.mult, mybir.AluOpType.add)
            nc.sync.dma_start(out_d[:, :, :], acc_sb[:])

    nc.compile()
    return nc


_CACHE = {}


def _get_program(cpc, edge_key, src, dst, repeats=1):
    key = (cpc, edge_key, repeats)
    if key not in _CACHE:
        plan = _Plan(src, dst, cpc)
        nc = _build(plan, 128, 64, 64, repeats=repeats)
        _CACHE[key] = (plan, nc)
    return _CACHE[key]


def _make_in_maps(plan, x, edge_index, W1, b1, W_mu, b_mu, W_log, b_log):
    import ml_dtypes

    x = np.asarray(x, np.float32)
    W1 = np.asarray(W1, np.float32)
    Wcat = np.concatenate([np.asarray(W_mu, np.float32),
                           np.asarray(W_log, np.float32)], axis=1)
    bcat = np.concatenate([np.asarray(b_mu, np.float32),
                           np.asarray(b_log, np.float32)])
    b1 = np.asarray(b1, np.float32)
    src = np.asarray(edge_index[0], np.int64)
    dst = np.asarray(edge_index[1], np.int64)
    n = x.shape[0]

    # xT bf16 in table layout [128, NPAD]
    xpad = np.zeros((NPAD, x.shape[1]), np.float32)
    for c in range(N_CORES):
        xpad[c * SPAD:c * SPAD + S] = x[c * S:(c + 1) * S]
    xt = np.ascontiguousarray(xpad.T).astype(ml_dtypes.bfloat16)

    deg = np.bincount(dst, minlength=n).astype(np.float32)
    dinv_full = np.zeros((P, N_CORES * T), np.float32)
    for c in range(N_CORES):
        dpad = np.zeros(SPAD, np.float32)
        dpad[:S] = 1.0 / np.sqrt(deg[c * S:(c + 1) * S] + 1.0)
        dinv_full[:, c * T:(c + 1) * T] = dpad.reshape(T, P).T

    w1_bf = np.zeros((W1.shape[0], P), ml_dtypes.bfloat16)
    w1_bf[:, :W1.shape[1]] = W1.astype(ml_dtypes.bfloat16)
    wc_pad = np.zeros((Wcat.shape[0], P), np.float32)
    wc_pad[:, :Wcat.shape[1]] = Wcat
    in_maps = []
    for c in range(N_CORES):
        idx_u, dl = plan.core_arrays(c, dst)
        in_maps.append({
            "xt": xt,
            "xt_own": np.ascontiguousarray(xt[:, c * SPAD:(c + 1) * SPAD]),
            "w1": w1_bf, "wcat": wc_pad, "b1": b1, "bcat": bcat,
            "dinv": dinv_full,
            "dinv_own": np.ascontiguousarray(
                dinv_full[:, c * T:(c + 1) * T]),
            "dstloc": dl, "gidx": idx_u,
        })
    return in_maps


def kernel(x, edge_index, W1, b1, W_mu, b_mu, W_log, b_log,
           cpc=8, _run_kwargs=None):
    from concourse.bass_utils import run_bass_kernel_spmd

    edge_index = np.asarray(edge_index)
    src = edge_index[0].astype(np.int64)
    dst = edge_index[1].astype(np.int64)
    edge_key = hash((src.tobytes(), dst.tobytes()))
    plan, nc = _get_program(cpc, edge_key, src, dst)
    in_maps = _make_in_maps(plan, x, edge_index, W1, b1, W_mu, b_mu,
                            W_log, b_log)

    global _LAST_RESULT, _LAST_IN_MAPS
    _LAST_IN_MAPS = in_maps
    res = run_bass_kernel_spmd(nc, in_maps, core_ids=list(range(N_CORES)),
                               **(_run_kwargs or {}))
    _LAST_RESULT = res
    lat = np.asarray(W_mu, np.float32).shape[1]
    outs = []
    for c in range(N_CORES):
        o = res.results[c]["out2"]        # [P, T, 64]
        o = o.transpose(1, 0, 2).reshape(SPAD, 64)[:S]
        outs.append(o)
    out = np.concatenate(outs, axis=0)
    return (out[:, :lat].copy(), out[:, lat:].copy())


_LAST_RESULT = None
_LAST_IN_MAPS = None


# revision 24
# speedup vs baseline: 1.0782x; 1.0782x over previous
"""GCN VGAE encoder (2-layer GCNConv, mu|logvar fused) on 8 TRN2 cores.

Strategy: shard destination nodes across 8 cores; partition edges by
dst (host-side); replicate weights. Layer-1 node table (x@W1)*dinv is
computed redundantly in full on every core (cheap on PE, kills one
AllGather); layer-2 table requires one fp32 AllGather. Per-edge
aggregation = dma_gather of 256B table rows from HBM (4 SWDGE queues,
small calls) + 0/1-matrix scatter matmuls into PSUM per dst tile.

Math: A_hat = D^-1/2 (A+I) D^-1/2 factorizes, so
    gcn(x, W) = dinv * [ (x@W)*dinv + A @ ((x@W)*dinv) ] + b
with dinv = 1/sqrt(deg+1). Self loop = the own-row term.

Table layout (both layers): [8 cores x 12544 rows x 64 f32] where core
section c row r (tile-padded: r = t*128 + p, 98 tiles, rows >= 12500
are pad) holds node c*12500 + r. Quarters of 25088 rows (= 2 core
sections) keep gather indices within int16. SPMD: the program is
identical on every core; all per-core differences are input data
(xt_own, dinv_own, gidx, dstloc).
"""

import os

import numpy as np

P = 128
N_CORES = 8
S = 12500             # real nodes per core
T = 98                # dst tiles per core
SPAD = T * P          # 12544 padded rows per core
NQ = 4                # table quarters (2 core sections each)
QR = 2 * SPAD         # 25088 rows per quarter
NPAD = N_CORES * SPAD


def _ceil_div(a, b):
    return -(-a // b)


class _Plan:
    """Host-side edge partitioning (SPMD: one common chunk structure =
    max over cores, padded)."""

    def __init__(self, src, dst, cpc):
        self.cpc = cpc

        core = dst // S                     # dst core
        drel = dst - core * S
        tt = drel // P                      # dst tile within core
        loc = (drel % P).astype(np.float32)
        csrc = src // S                     # src core
        q = csrc // 2                       # quarter of the table
        qsrc = ((csrc % 2) * SPAD + (src - csrc * S)).astype(np.int16)

        key = (core * NQ + q) * T + tt
        counts = np.bincount(key, minlength=N_CORES * NQ * T).reshape(
            N_CORES, NQ, T)
        # common run length per (quarter, dst tile): max over cores; runs
        # packed back-to-back per quarter stream (chunks may span tiles).
        self.rl = counts.max(axis=0)                         # [NQ, T]
        self.run_start = np.zeros((NQ, T), np.int64)
        self.run_start[:, 1:] = np.cumsum(self.rl, axis=1)[:, :-1]
        self.NQE = self.rl.sum(axis=1)                       # edges/quarter
        self.NQC = _ceil_div(self.NQE, P)                    # chunks/quarter
        self.NCH = int(self.NQC.sum())

        # order edges by (core, quarter, tile); rank within group
        sidx = np.lexsort((tt, q, core))
        self.sc = core[sidx]
        self.sq = q[sidx]
        self.st = tt[sidx]
        self.sqsrc = qsrc[sidx]
        self.sloc = loc[sidx]
        gkey = (self.sc * NQ + self.sq) * T + self.st
        first = np.r_[True, gkey[1:] != gkey[:-1]]
        gstart = np.flatnonzero(first)
        glen = np.diff(np.r_[gstart, len(gkey)])
        self.rank = np.arange(len(gkey)) - np.repeat(gstart, glen)

        # chunk-part map: tile t consumes chunks j0..j1 of each quarter's
        # stream; each (t, q, j) overlap owns one dstloc column.
        self.tile_ops = []        # [T] -> list of (q, j, cp_col)
        ncp = 0
        self.cp_base = np.zeros(T, np.int64)
        self.ops_before = np.zeros((NQ, T), np.int64)
        self.j0 = self.run_start // P
        for t in range(T):
            ops = []
            self.cp_base[t] = ncp
            acc = 0
            for qq in range(NQ):
                self.ops_before[qq, t] = acc
                r0 = int(self.run_start[qq, t])
                r1 = r0 + int(self.rl[qq, t])
                if r1 == r0:
                    continue
                for j in range(r0 // P, (r1 - 1) // P + 1):
                    ops.append((qq, j, ncp + acc + j - r0 // P))
                acc += (r1 - 1) // P - r0 // P + 1
            ncp += acc
            self.tile_ops.append(ops)
        self.NCP = ncp

        # gather calls per quarter (cpc chunks per call)
        self.ncalls = [_ceil_div(int(c), cpc) for c in self.NQC]
        self.call_col0 = {}
        col = 0
        for qq in range(NQ):
            for k in range(self.ncalls[qq]):
                L = min(cpc, int(self.NQC[qq]) - k * cpc)
                self.call_col0[(qq, k)] = (col, L)
                col += L * 8
        self.IDXCOLS = col

    def core_arrays(self, c, dst):
        """Per-core upload tensors: gather idx [128, IDXCOLS] i16,
        dstloc [128, NCP] f32."""
        cpc = self.cpc
        m_core = self.sc == c
        idx_out = np.zeros((P, self.IDXCOLS), np.int16)
        dl = np.full((self.NCP, P), 255.0, np.float32)
        mloc = self.sloc[m_core]
        mq = self.sq[m_core]
        mt = self.st[m_core]
        mrank = self.rank[m_core]
        msrc = self.sqsrc[m_core]
        pos = self.run_start[mq, mt] + mrank   # position in quarter stream
        cpcol = (self.cp_base[mt] + self.ops_before[mq, mt]
                 + pos // P - self.j0[mq, mt])
        dl[cpcol, pos % P] = mloc
        for qq in range(NQ):
            mm = mq == qq
            arr = np.zeros(int(self.NQC[qq]) * P, np.int16)
            arr[pos[mm]] = msrc[mm]
            for k in range(self.ncalls[qq]):
                c0, L = self.call_col0[(qq, k)]
                seg = arr[k * cpc * P:(k * cpc + L) * P]
                wrapped = seg.reshape(L * 8, 16).T       # [16, L*8]
                idx_out[:, c0:c0 + L * 8] = np.tile(wrapped, (8, 1))
        return idx_out, dl.T.copy()


def _build(plan, d_in, d_h, d_o, repeats=1):
    """Build the SPMD Bass program (identical on every core)."""
    import concourse.mybir as mybir
    import concourse.tile as tile
    from concourse import bacc
    from concourse.masks import make_identity

    F32 = mybir.dt.float32
    BF16 = mybir.dt.bfloat16
    I16 = mybir.dt.int16
    cpc = plan.cpc
    XC = SPAD // 2     # xT load chunk: quarter of a quarter (6272 rows)

    nc = bacc.Bacc("TRN2", target_bir_lowering=False, debug=False,
                   num_devices=N_CORES, num_swdge_queues=4)

    xt_d = nc.dram_tensor("xt", [d_in, NPAD], BF16, kind="ExternalInput")
    xto_d = nc.dram_tensor("xt_own", [d_in, SPAD], BF16,
                           kind="ExternalInput")
    w1_d = nc.dram_tensor("w1", [d_in, P], BF16, kind="ExternalInput")
    wc_d = nc.dram_tensor("wcat", [d_h, P], F32, kind="ExternalInput")
    b1_d = nc.dram_tensor("b1", [d_h], F32, kind="ExternalInput")
    bc_d = nc.dram_tensor("bcat", [d_o], F32, kind="ExternalInput")
    dinv_d = nc.dram_tensor("dinv", [P, N_CORES * T], F32,
                            kind="ExternalInput")
    dvo_d = nc.dram_tensor("dinv_own", [P, T], F32, kind="ExternalInput")
    dl_d = nc.dram_tensor("dstloc", [P, plan.NCP], F32, kind="ExternalInput")
    idx_d = nc.dram_tensor("gidx", [P, plan.IDXCOLS], I16,
                           kind="ExternalInput")
    out_d = nc.dram_tensor("out2", [P, T, d_o], F32, kind="ExternalOutput")

    t1q = [nc.dram_tensor(f"t1q{q}", [QR, P], BF16, kind="Internal")
           for q in range(NQ)]
    z2s_own = nc.dram_tensor("z2s_own", [SPAD, P], BF16, kind="Internal")
    z2s_full = nc.dram_tensor("z2s_full", [NPAD, P], BF16, kind="Internal",
                              addr_space="Shared")
    _priv = bool(os.environ.get("GCN_PRIV"))
    z2s_priv = (nc.dram_tensor("z2s_priv", [NPAD, P], BF16,
                               kind="Internal") if _priv else None)
    rg = [list(range(N_CORES))]

    _stage = int(os.environ.get("GCN_STAGE", "2"))
    _nocoll = bool(os.environ.get("GCN_NOCOLL"))
    _l1agg = os.environ.get("GCN_L1AGG", "1") != "0"
    _gbufs = int(os.environ.get("GCN_GBUFS", "8"))

    from contextlib import ExitStack

    with tile.TileContext(nc, num_cores=N_CORES) as tc, ExitStack() as st:
        cp = st.enter_context(tc.tile_pool(name="consts", bufs=1))
        bigp = st.enter_context(tc.tile_pool(name="big", bufs=1))
        xtp = st.enter_context(tc.tile_pool(name="xt", bufs=2))
        stp = st.enter_context(tc.tile_pool(name="stage", bufs=2))
        htp = st.enter_context(tc.tile_pool(name="ht", bufs=2))
        ohp = st.enter_context(tc.tile_pool(
            name="oh", bufs=int(os.environ.get("GCN_OBUFS", "6"))))
        gp = st.enter_context(tc.tile_pool(name="g", bufs=_gbufs))
        mmp = st.enter_context(tc.tile_pool(name="mm", bufs=4, space="PSUM"))
        aggp = st.enter_context(tc.tile_pool(name="agg", bufs=4,
                                             space="PSUM"))

        # ---- constants ----
        iota_i = cp.tile([P, P], mybir.dt.int32)
        nc.gpsimd.iota(iota_i[:], pattern=[[1, P]], base=0,
                       channel_multiplier=0)
        iota_f = cp.tile([P, P], F32)
        nc.vector.tensor_copy(iota_f[:], iota_i[:])
        ident = cp.tile([P, P], F32)
        make_identity(nc, ident[:])
        ident_bf = cp.tile([P, P], BF16)
        nc.vector.tensor_copy(ident_bf[:], ident[:])
        ones_row = cp.tile([1, P], F32)
        nc.gpsimd.memset(ones_row[:], 1.0)

        w1_sb = cp.tile([d_in, P], BF16)
        nc.sync.dma_start(w1_sb[:], w1_d[:, :])
        wc_sb = cp.tile([d_h, P], F32)
        nc.sync.dma_start(wc_sb[:], wc_d[:, :])
        b1r = cp.tile([1, d_h], F32)
        nc.sync.dma_start(b1r[:], b1_d[None, :])
        bcr = cp.tile([1, d_o], F32)
        nc.sync.dma_start(bcr[:], bc_d[None, :])

        b1bc = cp.tile([P, d_h], F32)
        ps = mmp.tile([P, P], F32, space="PSUM", tag="mm")
        nc.tensor.matmul(ps[:, :d_h], lhsT=ones_row[:], rhs=b1r[:],
                         start=True, stop=True)
        nc.vector.tensor_copy(b1bc[:], ps[:, :d_h])
        bcbc = cp.tile([P, d_o], F32)
        ps = mmp.tile([P, P], F32, space="PSUM", tag="mm")
        nc.tensor.matmul(ps[:, :d_o], lhsT=ones_row[:], rhs=bcr[:],
                         start=True, stop=True)
        nc.vector.tensor_copy(bcbc[:], ps[:, :d_o])

        dinv = cp.tile([P, N_CORES * T], F32)   # full-table dinv
        nc.sync.dma_start(dinv[:], dinv_d[:, :])
        dvo = cp.tile([P, T], F32)              # own dinv
        nc.sync.dma_start(dvo[:], dvo_d[:, :])
        dl_sb = cp.tile([P, plan.NCP], F32)
        nc.sync.dma_start(dl_sb[:], dl_d[:, :])
        idx_sb = cp.tile([P, plan.IDXCOLS], I16)
        nc.sync.dma_start(idx_sb[:], idx_d[:, :])

        acc_sb = bigp.tile([P, T, d_h], F32)
        z2s_sb = acc_sb    # acc is dead per-tile once the h-loop used it
        z2st_g = bigp.tile([P, T, P], BF16)

        OHK = int(os.environ.get("GCN_OHK", "16"))  # chunks per oh build

        def agg_pass(table_dram, acc, d_f):
            """acc[:, t, :] += sum_e onehot(dst) * table[src] (all 4
            quarters), via 4-queue HBM gathers."""
            issued = {}
            ohcache = {}
            callno = [0]

            def get_oh(cpcol):
                # batched one-hot build: one DVE op per OHK chunks, with
                # broadcast (stride-0) APs; built well ahead of the
                # consuming matmuls so the engines stay decoupled.
                B = cpcol // OHK
                if B not in ohcache:
                    from concourse.bass import broadcast_tensor_aps
                    L = min(OHK, plan.NCP - B * OHK)
                    ohb = ohp.tile([P, OHK, P], BF16, tag="ohb")
                    a0, a1 = broadcast_tensor_aps(
                        iota_f[:, None, :],
                        dl_sb[:, B * OHK:B * OHK + L, None])
                    nc.vector.tensor_tensor(ohb[:, :L, :], a0, a1,
                                            mybir.AluOpType.is_equal)
                    ohcache[B] = ohb
                return ohcache[B][:, cpcol - B * OHK, :]

            def get_call(qq, k):
                if (qq, k) not in issued:
                    c0, L = plan.call_col0[(qq, k)]
                    g = gp.tile([P, cpc, P], BF16, tag="g")
                    tab = (table_dram[qq] if isinstance(table_dram, list)
                           else table_dram[qq * QR:(qq + 1) * QR, :])
                    nc.gpsimd.dma_gather(
                        out_ap=g[:, :L, :],
                        in_ap=tab[:, :] if isinstance(table_dram, list)
                        else tab,
                        idxs_ap=idx_sb[:, c0:c0 + L * 8],
                        num_idxs=L * P,
                        num_idxs_reg=L * P,
                        elem_size=P,
                        single_packet=False,
                        queue_num=callno[0] % 4,
                    )
                    callno[0] += 1
                    issued[(qq, k)] = g
                return issued[(qq, k)]

            _noh = os.environ.get("GCN_NOOH")
            _fix = os.environ.get("GCN_FIXOH")
            for t in range(T):
                ops = plan.tile_ops[t]
                if not ops:
                    continue
                if _noh:
                    for i, (qq, j, cpcol) in enumerate(ops):
                        get_call(qq, j // cpc)
                    continue
                psa = aggp.tile([P, d_f], F32, space="PSUM", tag="agg")
                for i, (qq, j, cpcol) in enumerate(ops):
                    g = get_call(qq, j // cpc)
                    col = j % cpc
                    if _fix:
                        oh = ident_bf
                        get_oh(cpcol) if os.environ.get(
                            "GCN_OHCONST") else None
                    elif os.environ.get("GCN_OHONE"):
                        oh = get_oh(0)
                    else:
                        oh = get_oh(cpcol)
                    nc.tensor.matmul(psa[:], lhsT=oh[:],
                                     rhs=g[:, col, :d_f],
                                     start=(i == 0),
                                     stop=(i == len(ops) - 1))
                nc.vector.tensor_tensor(acc[:, t, :], acc[:, t, :],
                                        psa[:], mybir.AluOpType.add)

        for _rep in range(repeats):
            # ---- layer 1: full table computed locally, section-wise ----
            # t1s[r] = (x[r] @ W1) * dinv[r]
            for sec in range(N_CORES):
                stage = stp.tile([P, T, P], BF16, tag="stage")
                for h in range(2):       # two xT chunks per section
                    col0 = sec * SPAD + h * XC       # table row offset
                    xt = xtp.tile([d_in, XC], BF16, tag="xt")
                    nc.sync.dma_start(xt[:], xt_d[:, col0:col0 + XC])
                    for t in range(XC // P):
                        tc_glob = col0 // P + t      # global tile index
                        st_loc = h * (XC // P) + t   # tile within stage
                        psm = mmp.tile([P, P], F32, space="PSUM", tag="mm")
                        nc.tensor.matmul(
                            psm[:], lhsT=xt[:, t * P:(t + 1) * P],
                            rhs=w1_sb[:], start=True, stop=True)
                        nc.vector.tensor_scalar(
                            stage[:, st_loc, :], psm[:],
                            dinv[:, tc_glob:tc_glob + 1], None,
                            mybir.AluOpType.mult)
                nc.sync.dma_start(
                    t1q[sec // 2].rearrange("(q p) f -> p q f", p=P)[
                        :, (sec % 2) * T:(sec % 2 + 1) * T, :],
                    stage[:])

            # ---- seed acc with own (x@W1)*dinv section ----
            for h in range(2):
                xt = xtp.tile([d_in, XC], BF16, tag="xt")
                nc.sync.dma_start(xt[:], xto_d[:, h * XC:(h + 1) * XC])
                for t in range(XC // P):
                    tl = h * (XC // P) + t
                    psm = mmp.tile([P, P], F32, space="PSUM", tag="mm")
                    nc.tensor.matmul(
                        psm[:], lhsT=xt[:, t * P:(t + 1) * P],
                        rhs=w1_sb[:], start=True, stop=True)
                    nc.vector.tensor_scalar(
                        acc_sb[:, tl, :], psm[:, :d_h],
                        dvo[:, tl:tl + 1], None, mybir.AluOpType.mult)

            if _stage >= 1 and _l1agg:
                agg_pass(t1q, acc_sb, d_h)
            if os.environ.get("GCN_DUP_L1"):
                agg_pass(t1q, acc_sb, d_h)

            # ---- h = relu(acc * dinv_own + b1); z2s = (h @ Wcat)*dinv ----
            z2st = z2st_g
            for t in range(T):
                dv = dvo[:, t:t + 1]
                nc.vector.scalar_tensor_tensor(
                    acc_sb[:, t, :], acc_sb[:, t, :], dv, b1bc[:],
                    mybir.AluOpType.mult, mybir.AluOpType.add)
                nc.scalar.activation(acc_sb[:, t, :], acc_sb[:, t, :],
                                     mybir.ActivationFunctionType.Relu)
                pst = mmp.tile([P, P], F32, space="PSUM", tag="mm")
                nc.tensor.transpose(pst[:d_h, :], acc_sb[:, t, :], ident[:])
                hT = htp.tile([d_h, P], F32, tag="ht")
                nc.vector.tensor_copy(hT[:], pst[:d_h, :])
                psm = mmp.tile([P, P], F32, space="PSUM", tag="mm")
                nc.tensor.matmul(psm[:], lhsT=hT[:], rhs=wc_sb[:],
                                 start=True, stop=True)
                nc.vector.tensor_scalar(z2s_sb[:, t, :], psm[:, :d_o],
                                        dv, None, mybir.AluOpType.mult)
                nc.vector.tensor_scalar(z2st[:, t, :], psm[:],
                                        dv, None, mybir.AluOpType.mult)
            nc.sync.dma_start(
                z2s_own.rearrange("(t p) f -> p t f", p=P)[:, :, :],
                z2st[:])

            if _nocoll:
                nc.sync.dma_start(z2s_full[0:SPAD, :], z2s_own[:, :])
            else:
                nc.gpsimd.collective_compute(
                    "AllGather", mybir.AluOpType.bypass, replica_groups=rg,
                    ins=[z2s_own[:, :].opt()], outs=[z2s_full[:, :].opt()])

            # acc2 = z2s_sb (own section seed), aggregate in place
            if _stage >= 2:
                if _priv:
                    nc.sync.dma_start(z2s_priv[:, :], z2s_full[:, :])
                    agg_pass(z2s_priv, z2s_sb, d_o)
                else:
                    agg_pass(z2s_full, z2s_sb, d_o)

            # ---- out = acc2 * dinv_own + bcat (into acc_sb, then store) --
            for t in range(T):
                nc.vector.scalar_tensor_tensor(
                    acc_sb[:, t, :], z2s_sb[:, t, :],
                    dvo[:, t:t + 1], bcbc[:],
                    mybir.AluOpType.mult, mybir.AluOpType.add)
            nc.sync.dma_start(out_d[:, :, :], acc_sb[:])

    nc.compile()
    return nc


_CACHE = {}


def _get_program(cpc, edge_key, src, dst, repeats=1):
    key = (cpc, edge_key, repeats)
    if key not in _CACHE:
        plan = _Plan(src, dst, cpc)
        nc = _build(plan, 128, 64, 64, repeats=repeats)
        _CACHE[key] = (plan, nc)
    return _CACHE[key]


def _make_in_maps(plan, x, edge_index, W1, b1, W_mu, b_mu, W_log, b_log):
    import ml_dtypes

    x = np.asarray(x, np.float32)
    W1 = np.asarray(W1, np.float32)
    Wcat = np.concatenate([np.asarray(W_mu, np.float32),
                           np.asarray(W_log, np.float32)], axis=1)
    bcat = np.concatenate([np.asarray(b_mu, np.float32),
                           np.asarray(b_log, np.float32)])
    b1 = np.asarray(b1, np.float32)
    src = np.asarray(edge_index[0], np.int64)
    dst = np.asarray(edge_index[1], np.int64)
    n = x.shape[0]

    # xT bf16 in table layout [128, NPAD]
    xpad = np.zeros((NPAD, x.shape[1]), np.float32)
    for c in range(N_CORES):
        xpad[c * SPAD:c * SPAD + S] = x[c * S:(c + 1) * S]
    xt = np.ascontiguousarray(xpad.T).astype(ml_dtypes.bfloat16)

    deg = np.bincount(dst, minlength=n).astype(np.float32)
    dinv_full = np.zeros((P, N_CORES * T), np.float32)
    for c in range(N_CORES):
        dpad = np.zeros(SPAD, np.float32)
        dpad[:S] = 1.0 / np.sqrt(deg[c * S:(c + 1) * S] + 1.0)
        dinv_full[:, c * T:(c + 1) * T] = dpad.reshape(T, P).T

    w1_bf = np.zeros((W1.shape[0], P), ml_dtypes.bfloat16)
    w1_bf[:, :W1.shape[1]] = W1.astype(ml_dtypes.bfloat16)
    wc_pad = np.zeros((Wcat.shape[0], P), np.float32)
    wc_pad[:, :Wcat.shape[1]] = Wcat
    in_maps = []
    for c in range(N_CORES):
        idx_u, dl = plan.core_arrays(c, dst)
        in_maps.append({
            "xt": xt,
            "xt_own": np.ascontiguousarray(xt[:, c * SPAD:(c + 1) * SPAD]),
            "w1": w1_bf, "wcat": wc_pad, "b1": b1, "bcat": bcat,
            "dinv": dinv_full,
            "dinv_own": np.ascontiguousarray(
                dinv_full[:, c * T:(c + 1) * T]),
            "dstloc": dl, "gidx": idx_u,
        })
    return in_maps


def kernel(x, edge_index, W1, b1, W_mu, b_mu, W_log, b_log,
           cpc=8, _run_kwargs=None):
    from concourse.bass_utils import run_bass_kernel_spmd

    edge_index = np.asarray(edge_index)
    src = edge_index[0].astype(np.int64)
    dst = edge_index[1].astype(np.int64)
    edge_key = hash((src.tobytes(), dst.tobytes()))
    plan, nc = _get_program(cpc, edge_key, src, dst)
    in_maps = _make_in_maps(plan, x, edge_index, W1, b1, W_mu, b_mu,
                            W_log, b_log)

    global _LAST_RESULT, _LAST_IN_MAPS
    _LAST_IN_MAPS = in_maps
    res = run_bass_kernel_spmd(nc, in_maps, core_ids=list(range(N_CORES)),
                               **(_run_kwargs or {}))
    _LAST_RESULT = res
    lat = np.asarray(W_mu, np.float32).shape[1]
    outs = []
    for c in range(N_CORES):
        o = res.results[c]["out2"]        # [P, T, 64]
        o = o.transpose(1, 0, 2).reshape(SPAD, 64)[:S]
        outs.append(o)
    out = np.concatenate(outs, axis=0)
    return (out[:, :lat].copy(), out[:, lat:].copy())


_LAST_RESULT = None
_LAST_IN_MAPS = None
